# revision 37
# baseline (speedup 1.0000x reference)
"""Trainium2 Bass kernel: CapOnlyContrastiveLoss (margin contrastive loss, mean reduction).

reference math (N=8192, D=512, margin=0.2):
    scores[i,j]  = -||im_i - ex_j||        (via gemm identity)
    diag[i]      = -||im_i - s_i||         (only diag of l2_sim(im, s) is used)
    loss         = mean(relu(margin + scores - diag))

Strategy (v6): 4x2 core grid.  Host-side sharding stages each core's operands
in the layouts the engines want: fp8(-2*im)^T / fp8(ex)^T for the DoubleRow
matmuls (d-major, one 8KB descriptor per partition) and bf16 row-major copies
for the norm computations.  This removes every device-side XBAR transpose and
wide cast -- in v4/v5 those transposes saturated the DMA engines with 256-byte
packets and serialized the whole preprocessing pipeline.  Norms (imsq, dd,
exsq) are fused square+row-sum tensor_tensor_reduce ops; the exsq row layout
for the fold matmuls is built with one tiny [128,32] XBAR transpose + a
256B-descriptor DRAM bounce.  Epilogue min+accumulate is split across DVE /
ACT / GpSimd per GROUP_KIND to balance engine load.  A short warm-up matmul
chain keeps the PE HAM un-throttled until real matmuls flow.
"""

import numpy as np

import concourse.bacc as bacc
import concourse.bass as bass
import concourse.tile as tile
from concourse import bass_utils, mybir

N, D = 8192, 512
MARGIN = 0.2
P = 128
NJ = 512
GW = 2048
I_GROUPS, J_GROUPS = 4, 2
IM_R = N // I_GROUPS  # 2048
EX_R = N // J_GROUPS  # 4096
KC = D // P  # 4
N_IT = IM_R // P  # 16
N_JB = EX_R // GW  # 2
BANKS = GW // NJ  # 4
EX_PER_JB = GW // P  # 16
N_G = N_JB * N_IT  # 32

N_WARM = 18  # PE warm-up matmuls

# per-group epilogue kind: 'stt' (DVE scalar_tensor_tensor min+acc),
# 'act' (ACT relu(c-sq)+acc) -- ratio balances DVE vs ACT load
GROUP_KIND = ['stt', 'stt', 'stt', 'act'] * 8

F32 = mybir.dt.float32
BF16 = mybir.dt.bfloat16
FP8 = mybir.dt.float8e4
AF = mybir.ActivationFunctionType
ALU = mybir.AluOpType
DR = mybir.MatmulPerfMode.DoubleRow

NP_FP8 = mybir.dt.np(FP8)
NP_BF16 = mybir.dt.np(BF16)

_CACHE = {}


def _emit(tc, nc, im8_d, ex8_d, imb_d, sb_d, exb_d, acc_d, cvec_d):
    from contextlib import ExitStack

    with ExitStack() as ctx:
        singles = ctx.enter_context(tc.tile_pool(name="singles", bufs=1))
        nrm = ctx.enter_context(tc.tile_pool(name="nrm", bufs=3))
        scratch = ctx.enter_context(tc.tile_pool(name="scratch", bufs=4))
        sqp = ctx.enter_context(tc.tile_pool(name="sqp", bufs=3))
        psum = ctx.enter_context(tc.tile_pool(name="psum", bufs=2, space="PSUM"))
        dram = ctx.enter_context(tc.tile_pool(name="dram", bufs=1, space="DRAM"))

        imT8 = singles.tile([P, KC, IM_R], FP8)
        exT8s = [singles.tile([P, KC, GW], FP8, name=f"exT8_{j}") for j in range(N_JB)]
        imsq = singles.tile([P, N_IT], F32)
        ddv = singles.tile([P, N_IT], F32)
        cc = singles.tile([P, N_IT], F32)
        exsq_cols = [singles.tile([P, EX_PER_JB], F32, name=f"exsqc{j}")
                     for j in range(N_JB)]
        acc_sb = singles.tile([P, N_G], F32)
        onesb = singles.tile([P, P], BF16)
        zerosb = singles.tile([P, GW], BF16)
        exrowb = singles.tile([P, N_JB * GW], BF16)
        exrow_dram = dram.tile([N_JB, 2 * EX_PER_JB, P], BF16)

        nc.vector.memset(onesb, 1.0)
        nc.vector.memset(zerosb, 0.0)

        # ---- PE warm-up: WAW-chained dummy matmuls on one psum buffer ----
        ps_warm = psum.tile([P, GW], F32, tag="mm")
        for w in range(N_WARM):
            nc.tensor.matmul(ps_warm[:, 0:NJ], onesb, zerosb[:, 0:NJ],
                             start=True, stop=True)

        def emit_ex_chunk(jb, q, eng):
            # bf16 ex rows (u0..u0+3 within jb) -> exsq via square+rowsum,
            # on ACT (Square+accum) or DVE (STT x*x+accum) per `eng`.
            u0 = 4 * q
            exc = nrm.tile([P, 4, D], BF16, tag="exc", bufs=4)
            nc.sync.dma_start(out=exc, in_=exb_d[:, jb, u0:u0 + 4, :])
            for u in range(4):
                so = scratch.tile([P, D], BF16, tag="so", name="so")
                if eng == 'act':
                    nc.scalar.activation(
                        out=so, in_=exc[:, u, :], func=AF.Square,
                        accum_out=exsq_cols[jb][:, u0 + u:u0 + u + 1])
                else:
                    nc.vector.scalar_tensor_tensor(
                        out=so, in0=exc[:, u, :], scalar=1.0, in1=exc[:, u, :],
                        op0=ALU.mult, op1=ALU.mult,
                        accum_out=exsq_cols[jb][:, u0 + u:u0 + u + 1])

        def emit_exrow(jb):
            # hi/lo split of the exsq columns, tiny [128,32] XBAR transpose to
            # row layout, 256B-descriptor DRAM bounce to linearize, then
            # replicate to the 4 row-group base partitions for the fold MMs.
            # XBAR source must be [16k, 128m] -> pad the packed hi/lo to 128
            cols = exsq_cols[jb]
            packed = scratch.tile([P, P], BF16, tag="pk", name="pk")
            nc.vector.memset(packed, 0.0)
            pk2 = packed.rearrange("p (u h) -> p u h", h=2)
            nc.vector.tensor_copy(out=pk2[:, 0:EX_PER_JB, 0], in_=cols)
            nc.vector.tensor_tensor(out=pk2[:, 0:EX_PER_JB, 1], in0=cols,
                                    in1=pk2[:, 0:EX_PER_JB, 0], op=ALU.subtract)
            rowsT = scratch.tile([P, P], BF16, tag="rt", name="rt")
            nc.sync.dma_start_transpose(rowsT, packed)
            sl = slice(jb * GW, (jb + 1) * GW)
            nc.gpsimd.dma_start(out=exrow_dram[jb], in_=rowsT[0:2 * EX_PER_JB, :])
            for r in (0, 32, 64, 96):
                nc.gpsimd.dma_start(
                    out=exrowb[r:r + 2, sl].rearrange("h (u p) -> h u p", p=P),
                    in_=exrow_dram[jb].rearrange("(u h) p -> h u p", h=2))

        def emit_im_chunk(k):
            # bf16 im/s rows (tiles 4k..4k+3): imsq and dd = ||im-s||^2 via
            # DVE sub + fused square+rowsum.
            t0 = 4 * k
            imc = nrm.tile([P, 4, D], BF16, tag="imc")
            sc = nrm.tile([P, 4, D], BF16, tag="sc")
            nc.sync.dma_start(out=imc, in_=imb_d[:, t0:t0 + 4, :])
            nc.sync.dma_start(out=sc, in_=sb_d[:, t0:t0 + 4, :])
            dfc = nrm.tile([P, 4, D], BF16, tag="dfc")
            nc.vector.tensor_tensor(out=dfc, in0=imc, in1=sc, op=ALU.subtract)
            for u in range(4):
                t = t0 + u
                so = scratch.tile([P, D], BF16, tag="so", name="so")
                nc.vector.scalar_tensor_tensor(
                    out=so, in0=imc[:, u, :], scalar=1.0, in1=imc[:, u, :],
                    op0=ALU.mult, op1=ALU.mult,
                    accum_out=imsq[:, t:t + 1])
                so2 = scratch.tile([P, D], F32, tag="so2", name="so2")
                nc.vector.scalar_tensor_tensor(
                    out=so2, in0=dfc[:, u, :], scalar=1.0, in1=dfc[:, u, :],
                    op0=ALU.mult, op1=ALU.mult,
                    accum_out=ddv[:, t:t + 1])

        def emit_cc(k):
            # c = margin + sqrt(dd); emitted separately (after a group's sqrt)
            # so its ddv dependency never head-of-line blocks the ACT queue.
            t0 = 4 * k
            ccs = scratch.tile([P, 4], F32, tag="ccs")
            nc.scalar.activation(out=ccs, in_=ddv[:, t0:t0 + 4], func=AF.Sqrt)
            nc.vector.tensor_scalar_add(cc[:, t0:t0 + 4], ccs, MARGIN)

        def emit_group(g):
            jb, it = divmod(g, N_IT)
            ps = psum.tile([P, GW], F32, tag="mm")

            def mm_exsq(start, stop):
                for b in range(BANKS):
                    r = 32 * b
                    nc.tensor.matmul(
                        ps[:, b * NJ:(b + 1) * NJ],
                        onesb[r:r + 2, :],
                        exrowb[r:r + 2, jb * GW + b * NJ:jb * GW + (b + 1) * NJ],
                        start=start, stop=stop, tile_position=(r, 0))

            def mm_dr(c, start, stop):
                for b in range(BANKS):
                    nc.tensor.matmul(
                        ps[:, b * NJ:(b + 1) * NJ],
                        imT8[:, 2 * c:2 * c + 2, it * P:(it + 1) * P],
                        exT8s[jb][:, 2 * c:2 * c + 2, b * NJ:(b + 1) * NJ],
                        start=start, stop=stop, perf_mode=DR)

            if g < 2:
                # fold-last: DR matmuls stream without waiting for the exsq
                # row, so the PE never idles while the exrow chain finishes.
                mm_dr(0, True, False)
                mm_dr(1, False, False)
                mm_exsq(False, True)
            else:
                mm_exsq(True, False)
                mm_dr(0, False, False)
                mm_dr(1, False, True)

            sq = sqp.tile([P, GW], BF16, tag="sq")
            nc.scalar.activation(out=sq, in_=ps, func=AF.Sqrt,
                                 bias=imsq[:, it:it + 1], scale=1.0)
            # delay the min+accumulate of the PREVIOUS group until after this
            # group's sqrt, so the psum->sbuf sqrt (which frees the psum
            # buffer) is never queued behind an epilogue op.
            if pend[0] is not None:
                emit_min(*pend[0])
            pend[0] = (g, it, sq)

        def emit_min(g, it, sq):
            kind = GROUP_KIND[g]
            mout = sqp.tile([P, GW], BF16, tag="mout")
            if kind == 'act':
                nc.scalar.activation(
                    out=mout, in_=sq, func=AF.Relu,
                    bias=cc[:, it:it + 1], scale=-1.0,
                    accum_out=acc_sb[:, g:g + 1])
            else:
                nc.vector.scalar_tensor_tensor(
                    out=mout, in0=sq, scalar=cc[:, it:it + 1], in1=zerosb,
                    op0=ALU.min, op1=ALU.add,
                    accum_out=acc_sb[:, g:g + 1])

        pend = [None]

        # ---- preamble + main loop, interleaved so late preprocessing sits
        # behind early groups in every engine queue. ----
        emit_ex_chunk(0, 0, 'act')
        emit_ex_chunk(0, 1, 'act')
        emit_ex_chunk(0, 2, 'act')
        emit_ex_chunk(0, 3, 'act')
        nc.sync.dma_start(out=exT8s[0], in_=ex8_d[:, 0])
        nc.sync.dma_start(out=imT8, in_=im8_d)
        emit_exrow(0)
        emit_im_chunk(0)
        emit_group(0)
        emit_cc(0)
        emit_group(1)
        emit_im_chunk(1)
        emit_group(2)
        emit_cc(1)
        emit_group(3)
        nc.sync.dma_start(out=exT8s[1], in_=ex8_d[:, 1])
        emit_ex_chunk(1, 0, 'dve')
        emit_group(4)
        emit_group(5)
        emit_im_chunk(2)
        emit_group(6)
        emit_cc(2)
        emit_ex_chunk(1, 1, 'dve')
        emit_group(7)
        emit_group(8)
        emit_im_chunk(3)
        emit_group(9)
        emit_cc(3)
        emit_ex_chunk(1, 2, 'dve')
        emit_group(10)
        emit_ex_chunk(1, 3, 'dve')
        emit_group(11)
        emit_exrow(1)
        nc.sync.dma_start(out=cvec_d, in_=cc)
        for g in range(12, N_G):
            emit_group(g)
        emit_min(*pend[0])

        nc.sync.dma_start(out=acc_d, in_=acc_sb)


def build_program():
    nc = bacc.Bacc("TRN2", target_bir_lowering=False, debug=False)
    im8_d = nc.dram_tensor("im8", [P, KC, IM_R], FP8, kind="ExternalInput").ap()
    ex8_d = nc.dram_tensor("ex8", [P, N_JB, KC, GW], FP8, kind="ExternalInput").ap()
    imb_d = nc.dram_tensor("imb", [P, N_IT, D], BF16, kind="ExternalInput").ap()
    sb_d = nc.dram_tensor("sb", [P, N_IT, D], BF16, kind="ExternalInput").ap()
    exb_d = nc.dram_tensor("exb", [P, N_JB, EX_PER_JB, D], BF16,
                           kind="ExternalInput").ap()
    acc_d = nc.dram_tensor("acc", [P, N_G], F32, kind="ExternalOutput").ap()
    cvec_d = nc.dram_tensor("cvec", [P, N_IT], F32, kind="ExternalOutput").ap()
    with tile.TileContext(nc) as tc:
        _emit(tc, nc, im8_d, ex8_d, imb_d, sb_d, exb_d, acc_d, cvec_d)
    nc.compile()
    return nc


def get_program():
    if "nc" not in _CACHE:
        _CACHE["nc"] = build_program()
    return _CACHE["nc"]


def _prep_core(im_sl, s_sl, ex_sl):
    """Host-side layout/precision staging for one core.

    fp8 operands are d-major (transposed) so the DoubleRow matmuls read them
    directly; bf16 copies stay row-major for the norm computations.  Column
    order everywhere is the natural row index, so exsq/imsq/cc line up with
    the matmul output columns/partitions.
    """
    # [P, KC, IM_R]: element (p, kc, i) = -2*im[i, kc*128+p]
    im8 = np.ascontiguousarray(
        (-2.0 * im_sl.T).astype(NP_FP8).reshape(KC, P, IM_R).transpose(1, 0, 2))
    # [P, N_JB, KC, GW]: element (p, jb, kc, j) = ex[jb*GW+j, kc*128+p]
    ex8 = np.ascontiguousarray(
        ex_sl.T.astype(NP_FP8).reshape(KC, P, N_JB, GW).transpose(1, 2, 0, 3))
    # [P, N_IT, D]: row (it*128+p) at partition p, slot it
    imb = np.ascontiguousarray(
        im_sl.astype(NP_BF16).reshape(N_IT, P, D).transpose(1, 0, 2))
    sb = np.ascontiguousarray(
        s_sl.astype(NP_BF16).reshape(N_IT, P, D).transpose(1, 0, 2))
    # [P, N_JB, EX_PER_JB, D]: row (jb*GW + u*128 + p)
    exb = np.ascontiguousarray(
        ex_sl.astype(NP_BF16).reshape(N_JB, EX_PER_JB, P, D).transpose(2, 0, 1, 3))
    return {"im8": im8, "ex8": ex8, "imb": imb, "sb": sb, "exb": exb}


def make_in_maps(im, s, ex_s):
    in_maps = []
    for c in range(8):
        ig, jg = divmod(c, J_GROUPS)
        in_maps.append(_prep_core(
            np.asarray(im[ig * IM_R:(ig + 1) * IM_R], dtype=np.float32),
            np.asarray(s[ig * IM_R:(ig + 1) * IM_R], dtype=np.float32),
            np.asarray(ex_s[jg * EX_R:(jg + 1) * EX_R], dtype=np.float32)))
    return in_maps


def finish(results):
    total = 0.0
    for r in results:
        cvec = np.asarray(r["cvec"], dtype=np.float64)
        acc = np.asarray(r["acc"], dtype=np.float64)
        csum = cvec.sum(axis=0)
        for g in range(N_G):
            it = g % N_IT
            if GROUP_KIND[g] == 'act':
                total += acc[:, g].sum()
            else:
                total += GW * csum[it] - acc[:, g].sum()
    return np.array(total / (float(N) * float(N)), dtype=np.float32)


def kernel(im, s, ex_s):
    nc = get_program()
    res = bass_utils.run_bass_kernel_spmd(nc, make_in_maps(im, s, ex_s),
                                          core_ids=list(range(8)))
    return finish(res.results)


if __name__ == "__main__":
    rng = np.random.default_rng(0)
    im = rng.standard_normal((N, D), dtype=np.float32)
    s = rng.standard_normal((N, D), dtype=np.float32)
    ex = rng.standard_normal((N, D), dtype=np.float32)
    print(kernel(im, s, ex))


# revision 38
# speedup vs baseline: 1.1303x; 1.1303x over previous
"""Trainium2 Bass kernel: CapOnlyContrastiveLoss (margin contrastive loss, mean reduction).

reference math (N=8192, D=512, margin=0.2):
    scores[i,j]  = -||im_i - ex_j||        (via gemm identity)
    diag[i]      = -||im_i - s_i||         (only diag of l2_sim(im, s) is used)
    loss         = mean(relu(margin + scores - diag))

Strategy (v6): 4x2 core grid.  Host-side sharding stages each core's operands
in the layouts the engines want: fp8(-2*im)^T / fp8(ex)^T for the DoubleRow
matmuls (d-major, one 8KB descriptor per partition) and bf16 row-major copies
for the norm computations.  This removes every device-side XBAR transpose and
wide cast -- in v4/v5 those transposes saturated the DMA engines with 256-byte
packets and serialized the whole preprocessing pipeline.  Norms (imsq, dd,
exsq) are fused square+row-sum tensor_tensor_reduce ops; the exsq row layout
for the fold matmuls is built with one tiny [128,32] XBAR transpose + a
256B-descriptor DRAM bounce.  Epilogue min+accumulate is split across DVE /
ACT / GpSimd per GROUP_KIND to balance engine load.  A short warm-up matmul
chain keeps the PE HAM un-throttled until real matmuls flow.
"""

import numpy as np

import concourse.bacc as bacc
import concourse.bass as bass
import concourse.tile as tile
from concourse import bass_utils, mybir

N, D = 8192, 512
MARGIN = 0.2
P = 128
NJ = 512
GW = 2048
I_GROUPS, J_GROUPS = 4, 2
IM_R = N // I_GROUPS  # 2048
EX_R = N // J_GROUPS  # 4096
KC = D // P  # 4
N_IT = IM_R // P  # 16
N_JB = EX_R // GW  # 2
BANKS = GW // NJ  # 4
EX_PER_JB = GW // P  # 16
N_G = N_JB * N_IT  # 32

N_WARM = 24  # PE warm-up matmuls (spans the preamble so HAM stays at K=8/8)

# per-group epilogue kind: 'stt' (DVE scalar_tensor_tensor min+acc),
# 'act' (ACT relu(c-sq)+acc) -- ratio balances DVE vs ACT load
GROUP_KIND = ['stt', 'stt', 'stt', 'act'] * 8

F32 = mybir.dt.float32
BF16 = mybir.dt.bfloat16
FP8 = mybir.dt.float8e4
AF = mybir.ActivationFunctionType
ALU = mybir.AluOpType
DR = mybir.MatmulPerfMode.DoubleRow

NP_FP8 = mybir.dt.np(FP8)
NP_BF16 = mybir.dt.np(BF16)

_CACHE = {}


def _emit(tc, nc, im8_d, ex8_d, imb_d, sb_d, exb_d, acc_d, cvec_d):
    from contextlib import ExitStack

    with ExitStack() as ctx:
        singles = ctx.enter_context(tc.tile_pool(name="singles", bufs=1))
        nrm = ctx.enter_context(tc.tile_pool(name="nrm", bufs=3))
        scratch = ctx.enter_context(tc.tile_pool(name="scratch", bufs=4))
        sqp = ctx.enter_context(tc.tile_pool(name="sqp", bufs=3))
        psum = ctx.enter_context(tc.tile_pool(name="psum", bufs=2, space="PSUM"))
        dram = ctx.enter_context(tc.tile_pool(name="dram", bufs=1, space="DRAM"))

        imT8 = singles.tile([P, KC, IM_R], FP8)
        exT8s = [singles.tile([P, KC, GW], FP8, name=f"exT8_{j}") for j in range(N_JB)]
        imsq = singles.tile([P, N_IT], F32)
        ddv = singles.tile([P, N_IT], F32)
        cc = singles.tile([P, N_IT], F32)
        exsq_cols = [singles.tile([P, EX_PER_JB], F32, name=f"exsqc{j}")
                     for j in range(N_JB)]
        acc_sb = singles.tile([P, N_G], F32)
        onesb = singles.tile([P, P], BF16)
        zerosb = singles.tile([P, GW], BF16)
        exrowb = singles.tile([P, N_JB * GW], BF16)
        exrow_dram = dram.tile([N_JB, 2 * EX_PER_JB, P], BF16)

        nc.vector.memset(onesb, 1.0)
        nc.vector.memset(zerosb, 0.0)

        # ---- PE warm-up: WAW-chained dummy matmuls on one psum buffer ----
        ps_warm = psum.tile([P, GW], F32, tag="mm")
        for w in range(N_WARM):
            nc.tensor.matmul(ps_warm[:, 0:NJ], onesb, zerosb[:, 0:NJ],
                             start=True, stop=True)

        def emit_ex_chunk(jb, q, eng):
            # bf16 ex rows (u0..u0+3 within jb) -> exsq via square+rowsum,
            # on ACT (Square+accum) or DVE (STT x*x+accum) per `eng`.
            u0 = 4 * q
            exc = nrm.tile([P, 4, D], BF16, tag="exc", bufs=4)
            nc.sync.dma_start(out=exc, in_=exb_d[:, jb, u0:u0 + 4, :])
            for u in range(4):
                so = scratch.tile([P, D], BF16, tag="so", name="so")
                if eng == 'act':
                    nc.scalar.activation(
                        out=so, in_=exc[:, u, :], func=AF.Square,
                        accum_out=exsq_cols[jb][:, u0 + u:u0 + u + 1])
                else:
                    nc.vector.scalar_tensor_tensor(
                        out=so, in0=exc[:, u, :], scalar=1.0, in1=exc[:, u, :],
                        op0=ALU.mult, op1=ALU.mult,
                        accum_out=exsq_cols[jb][:, u0 + u:u0 + u + 1])

        def emit_exrow(jb):
            # hi/lo split of the exsq columns, tiny [128,32] XBAR transpose to
            # row layout, 256B-descriptor DRAM bounce to linearize, then
            # replicate to the 4 row-group base partitions for the fold MMs.
            # XBAR source must be [16k, 128m] -> pad the packed hi/lo to 128
            cols = exsq_cols[jb]
            packed = scratch.tile([P, P], BF16, tag="pk", name="pk")
            nc.vector.memset(packed, 0.0)
            pk2 = packed.rearrange("p (u h) -> p u h", h=2)
            nc.vector.tensor_copy(out=pk2[:, 0:EX_PER_JB, 0], in_=cols)
            nc.vector.tensor_tensor(out=pk2[:, 0:EX_PER_JB, 1], in0=cols,
                                    in1=pk2[:, 0:EX_PER_JB, 0], op=ALU.subtract)
            rowsT = scratch.tile([P, P], BF16, tag="rt", name="rt")
            nc.sync.dma_start_transpose(rowsT, packed)
            sl = slice(jb * GW, (jb + 1) * GW)
            nc.gpsimd.dma_start(out=exrow_dram[jb], in_=rowsT[0:2 * EX_PER_JB, :])
            for r in (0, 32, 64, 96):
                nc.gpsimd.dma_start(
                    out=exrowb[r:r + 2, sl].rearrange("h (u p) -> h u p", p=P),
                    in_=exrow_dram[jb].rearrange("(u h) p -> h u p", h=2))

        def emit_im_chunk(k):
            # bf16 im/s rows (tiles 4k..4k+3): imsq and dd = ||im-s||^2 via
            # DVE sub + fused square+rowsum.
            t0 = 4 * k
            imc = nrm.tile([P, 4, D], BF16, tag="imc")
            sc = nrm.tile([P, 4, D], BF16, tag="sc")
            nc.sync.dma_start(out=imc, in_=imb_d[:, t0:t0 + 4, :])
            nc.sync.dma_start(out=sc, in_=sb_d[:, t0:t0 + 4, :])
            dfc = nrm.tile([P, 4, D], BF16, tag="dfc")
            nc.vector.tensor_tensor(out=dfc, in0=imc, in1=sc, op=ALU.subtract)
            for u in range(4):
                t = t0 + u
                so = scratch.tile([P, D], BF16, tag="so", name="so")
                nc.vector.scalar_tensor_tensor(
                    out=so, in0=imc[:, u, :], scalar=1.0, in1=imc[:, u, :],
                    op0=ALU.mult, op1=ALU.mult,
                    accum_out=imsq[:, t:t + 1])
                so2 = scratch.tile([P, D], F32, tag="so2", name="so2")
                nc.vector.scalar_tensor_tensor(
                    out=so2, in0=dfc[:, u, :], scalar=1.0, in1=dfc[:, u, :],
                    op0=ALU.mult, op1=ALU.mult,
                    accum_out=ddv[:, t:t + 1])

        def emit_cc(k):
            # c = margin + sqrt(dd); emitted separately (after a group's sqrt)
            # so its ddv dependency never head-of-line blocks the ACT queue.
            t0 = 4 * k
            ccs = scratch.tile([P, 4], F32, tag="ccs")
            nc.scalar.activation(out=ccs, in_=ddv[:, t0:t0 + 4], func=AF.Sqrt)
            nc.vector.tensor_scalar_add(cc[:, t0:t0 + 4], ccs, MARGIN)

        def emit_group(g):
            jb, it = divmod(g, N_IT)
            ps = psum.tile([P, GW], F32, tag="mm")

            def mm_exsq(start, stop):
                for b in range(BANKS):
                    r = 32 * b
                    nc.tensor.matmul(
                        ps[:, b * NJ:(b + 1) * NJ],
                        onesb[r:r + 2, :],
                        exrowb[r:r + 2, jb * GW + b * NJ:jb * GW + (b + 1) * NJ],
                        start=start, stop=stop, tile_position=(r, 0))

            def mm_dr(c, start, stop):
                for b in range(BANKS):
                    nc.tensor.matmul(
                        ps[:, b * NJ:(b + 1) * NJ],
                        imT8[:, 2 * c:2 * c + 2, it * P:(it + 1) * P],
                        exT8s[jb][:, 2 * c:2 * c + 2, b * NJ:(b + 1) * NJ],
                        start=start, stop=stop, perf_mode=DR)

            if g < 2:
                # fold-last: DR matmuls stream without waiting for the exsq
                # row, so the PE never idles while the exrow chain finishes.
                mm_dr(0, True, False)
                mm_dr(1, False, False)
                mm_exsq(False, True)
            else:
                mm_exsq(True, False)
                mm_dr(0, False, False)
                mm_dr(1, False, True)

            sq = sqp.tile([P, GW], BF16, tag="sq")
            nc.scalar.activation(out=sq, in_=ps, func=AF.Sqrt,
                                 bias=imsq[:, it:it + 1], scale=1.0)
            # delay the min+accumulate of the PREVIOUS group until after this
            # group's sqrt, so the psum->sbuf sqrt (which frees the psum
            # buffer) is never queued behind an epilogue op.
            if pend[0] is not None:
                emit_min(*pend[0])
            pend[0] = (g, it, sq)

        def emit_min(g, it, sq):
            kind = GROUP_KIND[g]
            mout = sqp.tile([P, GW], BF16, tag="mout")
            if kind == 'act':
                nc.scalar.activation(
                    out=mout, in_=sq, func=AF.Relu,
                    bias=cc[:, it:it + 1], scale=-1.0,
                    accum_out=acc_sb[:, g:g + 1])
            else:
                nc.vector.scalar_tensor_tensor(
                    out=mout, in0=sq, scalar=cc[:, it:it + 1], in1=zerosb,
                    op0=ALU.min, op1=ALU.add,
                    accum_out=acc_sb[:, g:g + 1])

        pend = [None]

        # ---- preamble + main loop, interleaved so late preprocessing sits
        # behind early groups in every engine queue. ----
        emit_ex_chunk(0, 0, 'act')
        emit_ex_chunk(0, 1, 'act')
        emit_ex_chunk(0, 2, 'act')
        emit_ex_chunk(0, 3, 'act')
        nc.sync.dma_start(out=exT8s[0], in_=ex8_d[:, 0])
        nc.sync.dma_start(out=imT8, in_=im8_d)
        emit_exrow(0)
        emit_im_chunk(0)
        emit_group(0)
        emit_cc(0)
        emit_group(1)
        emit_im_chunk(1)
        emit_group(2)
        emit_cc(1)
        emit_group(3)
        nc.sync.dma_start(out=exT8s[1], in_=ex8_d[:, 1])
        emit_ex_chunk(1, 0, 'dve')
        emit_group(4)
        emit_group(5)
        emit_im_chunk(2)
        emit_group(6)
        emit_cc(2)
        emit_ex_chunk(1, 1, 'dve')
        emit_group(7)
        emit_group(8)
        emit_im_chunk(3)
        emit_group(9)
        emit_cc(3)
        emit_ex_chunk(1, 2, 'dve')
        emit_group(10)
        emit_ex_chunk(1, 3, 'dve')
        emit_group(11)
        emit_exrow(1)
        nc.sync.dma_start(out=cvec_d, in_=cc)
        for g in range(12, N_G):
            emit_group(g)
        emit_min(*pend[0])

        nc.sync.dma_start(out=acc_d, in_=acc_sb)


def build_program():
    nc = bacc.Bacc("TRN2", target_bir_lowering=False, debug=False)
    im8_d = nc.dram_tensor("im8", [P, KC, IM_R], FP8, kind="ExternalInput").ap()
    ex8_d = nc.dram_tensor("ex8", [P, N_JB, KC, GW], FP8, kind="ExternalInput").ap()
    imb_d = nc.dram_tensor("imb", [P, N_IT, D], BF16, kind="ExternalInput").ap()
    sb_d = nc.dram_tensor("sb", [P, N_IT, D], BF16, kind="ExternalInput").ap()
    exb_d = nc.dram_tensor("exb", [P, N_JB, EX_PER_JB, D], BF16,
                           kind="ExternalInput").ap()
    acc_d = nc.dram_tensor("acc", [P, N_G], F32, kind="ExternalOutput").ap()
    cvec_d = nc.dram_tensor("cvec", [P, N_IT], F32, kind="ExternalOutput").ap()
    with tile.TileContext(nc) as tc:
        _emit(tc, nc, im8_d, ex8_d, imb_d, sb_d, exb_d, acc_d, cvec_d)
    nc.compile()
    return nc


def get_program():
    if "nc" not in _CACHE:
        _CACHE["nc"] = build_program()
    return _CACHE["nc"]


def _prep_core(im_sl, s_sl, ex_sl):
    """Host-side layout/precision staging for one core.

    fp8 operands are d-major (transposed) so the DoubleRow matmuls read them
    directly; bf16 copies stay row-major for the norm computations.  Column
    order everywhere is the natural row index, so exsq/imsq/cc line up with
    the matmul output columns/partitions.
    """
    # [P, KC, IM_R]: element (p, kc, i) = -2*im[i, kc*128+p]
    im8 = np.ascontiguousarray(
        (-2.0 * im_sl.T).astype(NP_FP8).reshape(KC, P, IM_R).transpose(1, 0, 2))
    # [P, N_JB, KC, GW]: element (p, jb, kc, j) = ex[jb*GW+j, kc*128+p]
    ex8 = np.ascontiguousarray(
        ex_sl.T.astype(NP_FP8).reshape(KC, P, N_JB, GW).transpose(1, 2, 0, 3))
    # [P, N_IT, D]: row (it*128+p) at partition p, slot it
    imb = np.ascontiguousarray(
        im_sl.astype(NP_BF16).reshape(N_IT, P, D).transpose(1, 0, 2))
    sb = np.ascontiguousarray(
        s_sl.astype(NP_BF16).reshape(N_IT, P, D).transpose(1, 0, 2))
    # [P, N_JB, EX_PER_JB, D]: row (jb*GW + u*128 + p)
    exb = np.ascontiguousarray(
        ex_sl.astype(NP_BF16).reshape(N_JB, EX_PER_JB, P, D).transpose(2, 0, 1, 3))
    return {"im8": im8, "ex8": ex8, "imb": imb, "sb": sb, "exb": exb}


def make_in_maps(im, s, ex_s):
    in_maps = []
    for c in range(8):
        ig, jg = divmod(c, J_GROUPS)
        in_maps.append(_prep_core(
            np.asarray(im[ig * IM_R:(ig + 1) * IM_R], dtype=np.float32),
            np.asarray(s[ig * IM_R:(ig + 1) * IM_R], dtype=np.float32),
            np.asarray(ex_s[jg * EX_R:(jg + 1) * EX_R], dtype=np.float32)))
    return in_maps


def finish(results):
    total = 0.0
    for r in results:
        cvec = np.asarray(r["cvec"], dtype=np.float64)
        acc = np.asarray(r["acc"], dtype=np.float64)
        csum = cvec.sum(axis=0)
        for g in range(N_G):
            it = g % N_IT
            if GROUP_KIND[g] == 'act':
                total += acc[:, g].sum()
            else:
                total += GW * csum[it] - acc[:, g].sum()
    return np.array(total / (float(N) * float(N)), dtype=np.float32)


def kernel(im, s, ex_s):
    nc = get_program()
    res = bass_utils.run_bass_kernel_spmd(nc, make_in_maps(im, s, ex_s),
                                          core_ids=list(range(8)))
    return finish(res.results)


if __name__ == "__main__":
    rng = np.random.default_rng(0)
    im = rng.standard_normal((N, D), dtype=np.float32)
    s = rng.standard_normal((N, D), dtype=np.float32)
    ex = rng.standard_normal((N, D), dtype=np.float32)
    print(kernel(im, s, ex))


# revision 39
# speedup vs baseline: 1.1513x; 1.0185x over previous
"""Trainium2 Bass kernel: CapOnlyContrastiveLoss (margin contrastive loss, mean reduction).

reference math (N=8192, D=512, margin=0.2):
    scores[i,j]  = -||im_i - ex_j||        (via gemm identity)
    diag[i]      = -||im_i - s_i||         (only diag of l2_sim(im, s) is used)
    loss         = mean(relu(margin + scores - diag))

Strategy (v6): 4x2 core grid.  Host-side sharding stages each core's operands
in the layouts the engines want: fp8(-2*im)^T / fp8(ex)^T for the DoubleRow
matmuls (d-major, one 8KB descriptor per partition) and bf16 row-major copies
for the norm computations.  This removes every device-side XBAR transpose and
wide cast -- in v4/v5 those transposes saturated the DMA engines with 256-byte
packets and serialized the whole preprocessing pipeline.  Norms (imsq, dd,
exsq) are fused square+row-sum tensor_tensor_reduce ops; the exsq row layout
for the fold matmuls is built with one tiny [128,32] XBAR transpose + a
256B-descriptor DRAM bounce.  Epilogue min+accumulate is split across DVE /
ACT / GpSimd per GROUP_KIND to balance engine load.  A short warm-up matmul
chain keeps the PE HAM un-throttled until real matmuls flow.
"""

import numpy as np

import concourse.bacc as bacc
import concourse.bass as bass
import concourse.tile as tile
from concourse import bass_utils, mybir

N, D = 8192, 512
MARGIN = 0.2
P = 128
NJ = 512
GW = 2048
I_GROUPS, J_GROUPS = 4, 2
IM_R = N // I_GROUPS  # 2048
EX_R = N // J_GROUPS  # 4096
KC = D // P  # 4
N_IT = IM_R // P  # 16
N_JB = EX_R // GW  # 2
BANKS = GW // NJ  # 4
EX_PER_JB = GW // P  # 16
N_G = N_JB * N_IT  # 32

N_WARM = 24  # PE warm-up matmuls (spans the preamble so HAM stays at K=8/8)

# per-group epilogue kind: 'stt' (DVE scalar_tensor_tensor min+acc),
# 'act' (ACT relu(c-sq)+acc) -- ratio balances DVE vs ACT load
# early half: DVE also runs norm squares -> 1-in-4 groups on ACT;
# late half: ACT is the window binder -> thin to 1-in-8.
GROUP_KIND = (['stt', 'stt', 'stt', 'act'] * 4 +
              ['stt', 'stt', 'stt', 'stt', 'stt', 'stt', 'stt', 'act'] * 2)

F32 = mybir.dt.float32
BF16 = mybir.dt.bfloat16
FP8 = mybir.dt.float8e4
AF = mybir.ActivationFunctionType
ALU = mybir.AluOpType
DR = mybir.MatmulPerfMode.DoubleRow

NP_FP8 = mybir.dt.np(FP8)
NP_BF16 = mybir.dt.np(BF16)

_CACHE = {}


def _emit(tc, nc, im8_d, ex8_d, imb_d, sb_d, exb_d, acc_d, cvec_d):
    from contextlib import ExitStack

    with ExitStack() as ctx:
        singles = ctx.enter_context(tc.tile_pool(name="singles", bufs=1))
        nrm = ctx.enter_context(tc.tile_pool(name="nrm", bufs=3))
        scratch = ctx.enter_context(tc.tile_pool(name="scratch", bufs=4))
        sqp = ctx.enter_context(tc.tile_pool(name="sqp", bufs=3))
        psum = ctx.enter_context(tc.tile_pool(name="psum", bufs=2, space="PSUM"))
        dram = ctx.enter_context(tc.tile_pool(name="dram", bufs=1, space="DRAM"))

        imT8 = singles.tile([P, KC, IM_R], FP8)
        exT8s = [singles.tile([P, KC, GW], FP8, name=f"exT8_{j}") for j in range(N_JB)]
        imsq = singles.tile([P, N_IT], F32)
        ddv = singles.tile([P, N_IT], F32)
        cc = singles.tile([P, N_IT], F32)
        exsq_cols = [singles.tile([P, EX_PER_JB], F32, name=f"exsqc{j}")
                     for j in range(N_JB)]
        acc_sb = singles.tile([P, N_G], F32)
        onesb = singles.tile([P, P], BF16)
        zerosb = singles.tile([P, GW], BF16)
        exrowb = singles.tile([P, N_JB * GW], BF16)
        exrow_dram = dram.tile([N_JB, 2 * EX_PER_JB, P], BF16)

        nc.vector.memset(onesb, 1.0)
        nc.vector.memset(zerosb, 0.0)

        # ---- PE warm-up: WAW-chained dummy matmuls on one psum buffer ----
        ps_warm = psum.tile([P, GW], F32, tag="mm")
        for w in range(N_WARM):
            nc.tensor.matmul(ps_warm[:, 0:NJ], onesb, zerosb[:, 0:NJ],
                             start=True, stop=True)

        def emit_ex_chunk(jb, q, eng):
            # bf16 ex rows (u0..u0+3 within jb) -> exsq via square+rowsum,
            # on ACT (Square+accum) or DVE (STT x*x+accum) per `eng`.
            u0 = 4 * q
            exc = nrm.tile([P, 4, D], BF16, tag="exc", bufs=4)
            nc.sync.dma_start(out=exc, in_=exb_d[:, jb, u0:u0 + 4, :])
            for u in range(4):
                so = scratch.tile([P, D], BF16, tag="so", name="so")
                if eng == 'act':
                    nc.scalar.activation(
                        out=so, in_=exc[:, u, :], func=AF.Square,
                        accum_out=exsq_cols[jb][:, u0 + u:u0 + u + 1])
                else:
                    nc.vector.scalar_tensor_tensor(
                        out=so, in0=exc[:, u, :], scalar=1.0, in1=exc[:, u, :],
                        op0=ALU.mult, op1=ALU.mult,
                        accum_out=exsq_cols[jb][:, u0 + u:u0 + u + 1])

        def emit_exrow(jb):
            # hi/lo split of the exsq columns, tiny [128,32] XBAR transpose to
            # row layout, 256B-descriptor DRAM bounce to linearize, then
            # replicate to the 4 row-group base partitions for the fold MMs.
            # XBAR source must be [16k, 128m] -> pad the packed hi/lo to 128
            cols = exsq_cols[jb]
            packed = scratch.tile([P, P], BF16, tag="pk", name="pk")
            nc.vector.memset(packed, 0.0)
            pk2 = packed.rearrange("p (u h) -> p u h", h=2)
            nc.vector.tensor_copy(out=pk2[:, 0:EX_PER_JB, 0], in_=cols)
            nc.vector.tensor_tensor(out=pk2[:, 0:EX_PER_JB, 1], in0=cols,
                                    in1=pk2[:, 0:EX_PER_JB, 0], op=ALU.subtract)
            rowsT = scratch.tile([P, P], BF16, tag="rt", name="rt")
            nc.sync.dma_start_transpose(rowsT, packed)
            sl = slice(jb * GW, (jb + 1) * GW)
            nc.gpsimd.dma_start(out=exrow_dram[jb], in_=rowsT[0:2 * EX_PER_JB, :])
            for r in (0, 32, 64, 96):
                nc.gpsimd.dma_start(
                    out=exrowb[r:r + 2, sl].rearrange("h (u p) -> h u p", p=P),
                    in_=exrow_dram[jb].rearrange("(u h) p -> h u p", h=2))

        def emit_im_chunk(k):
            # bf16 im/s rows (tiles 4k..4k+3): imsq and dd = ||im-s||^2 via
            # DVE sub + fused square+rowsum.
            t0 = 4 * k
            imc = nrm.tile([P, 4, D], BF16, tag="imc")
            sc = nrm.tile([P, 4, D], BF16, tag="sc")
            nc.sync.dma_start(out=imc, in_=imb_d[:, t0:t0 + 4, :])
            nc.sync.dma_start(out=sc, in_=sb_d[:, t0:t0 + 4, :])
            dfc = nrm.tile([P, 4, D], BF16, tag="dfc")
            nc.vector.tensor_tensor(out=dfc, in0=imc, in1=sc, op=ALU.subtract)
            for u in range(4):
                t = t0 + u
                so = scratch.tile([P, D], BF16, tag="so", name="so")
                nc.vector.scalar_tensor_tensor(
                    out=so, in0=imc[:, u, :], scalar=1.0, in1=imc[:, u, :],
                    op0=ALU.mult, op1=ALU.mult,
                    accum_out=imsq[:, t:t + 1])
                so2 = scratch.tile([P, D], F32, tag="so2", name="so2")
                nc.vector.scalar_tensor_tensor(
                    out=so2, in0=dfc[:, u, :], scalar=1.0, in1=dfc[:, u, :],
                    op0=ALU.mult, op1=ALU.mult,
                    accum_out=ddv[:, t:t + 1])

        def emit_cc(k):
            # c = margin + sqrt(dd); emitted separately (after a group's sqrt)
            # so its ddv dependency never head-of-line blocks the ACT queue.
            t0 = 4 * k
            ccs = scratch.tile([P, 4], F32, tag="ccs")
            nc.scalar.activation(out=ccs, in_=ddv[:, t0:t0 + 4], func=AF.Sqrt)
            nc.vector.tensor_scalar_add(cc[:, t0:t0 + 4], ccs, MARGIN)

        def emit_group(g):
            jb, it = divmod(g, N_IT)
            ps = psum.tile([P, GW], F32, tag="mm")

            def mm_exsq(start, stop):
                for b in range(BANKS):
                    r = 32 * b
                    nc.tensor.matmul(
                        ps[:, b * NJ:(b + 1) * NJ],
                        onesb[r:r + 2, :],
                        exrowb[r:r + 2, jb * GW + b * NJ:jb * GW + (b + 1) * NJ],
                        start=start, stop=stop, tile_position=(r, 0))

            def mm_dr(c, start, stop):
                for b in range(BANKS):
                    nc.tensor.matmul(
                        ps[:, b * NJ:(b + 1) * NJ],
                        imT8[:, 2 * c:2 * c + 2, it * P:(it + 1) * P],
                        exT8s[jb][:, 2 * c:2 * c + 2, b * NJ:(b + 1) * NJ],
                        start=start, stop=stop, perf_mode=DR)

            if g < 2:
                # fold-last: DR matmuls stream without waiting for the exsq
                # row, so the PE never idles while the exrow chain finishes.
                mm_dr(0, True, False)
                mm_dr(1, False, False)
                mm_exsq(False, True)
            else:
                mm_exsq(True, False)
                mm_dr(0, False, False)
                mm_dr(1, False, True)

            sq = sqp.tile([P, GW], BF16, tag="sq")
            nc.scalar.activation(out=sq, in_=ps, func=AF.Sqrt,
                                 bias=imsq[:, it:it + 1], scale=1.0)
            # delay the min+accumulate of the PREVIOUS group until after this
            # group's sqrt, so the psum->sbuf sqrt (which frees the psum
            # buffer) is never queued behind an epilogue op.
            if pend[0] is not None:
                emit_min(*pend[0])
            pend[0] = (g, it, sq)

        def emit_min(g, it, sq):
            kind = GROUP_KIND[g]
            mout = sqp.tile([P, GW], BF16, tag="mout")
            if kind == 'act':
                nc.scalar.activation(
                    out=mout, in_=sq, func=AF.Relu,
                    bias=cc[:, it:it + 1], scale=-1.0,
                    accum_out=acc_sb[:, g:g + 1])
            else:
                nc.vector.scalar_tensor_tensor(
                    out=mout, in0=sq, scalar=cc[:, it:it + 1], in1=zerosb,
                    op0=ALU.min, op1=ALU.add,
                    accum_out=acc_sb[:, g:g + 1])

        pend = [None]

        # ---- preamble + main loop, interleaved so late preprocessing sits
        # behind early groups in every engine queue. ----
        emit_ex_chunk(0, 0, 'act')
        emit_ex_chunk(0, 1, 'act')
        emit_ex_chunk(0, 2, 'act')
        emit_ex_chunk(0, 3, 'act')
        nc.sync.dma_start(out=exT8s[0], in_=ex8_d[:, 0])
        nc.sync.dma_start(out=imT8, in_=im8_d)
        emit_exrow(0)
        emit_im_chunk(0)
        emit_group(0)
        emit_cc(0)
        emit_group(1)
        emit_im_chunk(1)
        emit_group(2)
        emit_cc(1)
        emit_group(3)
        nc.sync.dma_start(out=exT8s[1], in_=ex8_d[:, 1])
        emit_ex_chunk(1, 0, 'dve')
        emit_group(4)
        emit_group(5)
        emit_im_chunk(2)
        emit_group(6)
        emit_cc(2)
        emit_ex_chunk(1, 1, 'dve')
        emit_group(7)
        emit_group(8)
        emit_im_chunk(3)
        emit_group(9)
        emit_cc(3)
        emit_ex_chunk(1, 2, 'dve')
        emit_group(10)
        emit_ex_chunk(1, 3, 'dve')
        emit_group(11)
        emit_exrow(1)
        nc.sync.dma_start(out=cvec_d, in_=cc)
        for g in range(12, N_G):
            emit_group(g)
        emit_min(*pend[0])

        nc.sync.dma_start(out=acc_d, in_=acc_sb)


def build_program():
    nc = bacc.Bacc("TRN2", target_bir_lowering=False, debug=False)
    im8_d = nc.dram_tensor("im8", [P, KC, IM_R], FP8, kind="ExternalInput").ap()
    ex8_d = nc.dram_tensor("ex8", [P, N_JB, KC, GW], FP8, kind="ExternalInput").ap()
    imb_d = nc.dram_tensor("imb", [P, N_IT, D], BF16, kind="ExternalInput").ap()
    sb_d = nc.dram_tensor("sb", [P, N_IT, D], BF16, kind="ExternalInput").ap()
    exb_d = nc.dram_tensor("exb", [P, N_JB, EX_PER_JB, D], BF16,
                           kind="ExternalInput").ap()
    acc_d = nc.dram_tensor("acc", [P, N_G], F32, kind="ExternalOutput").ap()
    cvec_d = nc.dram_tensor("cvec", [P, N_IT], F32, kind="ExternalOutput").ap()
    with tile.TileContext(nc) as tc:
        _emit(tc, nc, im8_d, ex8_d, imb_d, sb_d, exb_d, acc_d, cvec_d)
    nc.compile()
    return nc


def get_program():
    if "nc" not in _CACHE:
        _CACHE["nc"] = build_program()
    return _CACHE["nc"]


def _prep_core(im_sl, s_sl, ex_sl):
    """Host-side layout/precision staging for one core.

    fp8 operands are d-major (transposed) so the DoubleRow matmuls read them
    directly; bf16 copies stay row-major for the norm computations.  Column
    order everywhere is the natural row index, so exsq/imsq/cc line up with
    the matmul output columns/partitions.
    """
    # [P, KC, IM_R]: element (p, kc, i) = -2*im[i, kc*128+p]
    im8 = np.ascontiguousarray(
        (-2.0 * im_sl.T).astype(NP_FP8).reshape(KC, P, IM_R).transpose(1, 0, 2))
    # [P, N_JB, KC, GW]: element (p, jb, kc, j) = ex[jb*GW+j, kc*128+p]
    ex8 = np.ascontiguousarray(
        ex_sl.T.astype(NP_FP8).reshape(KC, P, N_JB, GW).transpose(1, 2, 0, 3))
    # [P, N_IT, D]: row (it*128+p) at partition p, slot it
    imb = np.ascontiguousarray(
        im_sl.astype(NP_BF16).reshape(N_IT, P, D).transpose(1, 0, 2))
    sb = np.ascontiguousarray(
        s_sl.astype(NP_BF16).reshape(N_IT, P, D).transpose(1, 0, 2))
    # [P, N_JB, EX_PER_JB, D]: row (jb*GW + u*128 + p)
    exb = np.ascontiguousarray(
        ex_sl.astype(NP_BF16).reshape(N_JB, EX_PER_JB, P, D).transpose(2, 0, 1, 3))
    return {"im8": im8, "ex8": ex8, "imb": imb, "sb": sb, "exb": exb}


def make_in_maps(im, s, ex_s):
    in_maps = []
    for c in range(8):
        ig, jg = divmod(c, J_GROUPS)
        in_maps.append(_prep_core(
            np.asarray(im[ig * IM_R:(ig + 1) * IM_R], dtype=np.float32),
            np.asarray(s[ig * IM_R:(ig + 1) * IM_R], dtype=np.float32),
            np.asarray(ex_s[jg * EX_R:(jg + 1) * EX_R], dtype=np.float32)))
    return in_maps


def finish(results):
    total = 0.0
    for r in results:
        cvec = np.asarray(r["cvec"], dtype=np.float64)
        acc = np.asarray(r["acc"], dtype=np.float64)
        csum = cvec.sum(axis=0)
        for g in range(N_G):
            it = g % N_IT
            if GROUP_KIND[g] == 'act':
                total += acc[:, g].sum()
            else:
                total += GW * csum[it] - acc[:, g].sum()
    return np.array(total / (float(N) * float(N)), dtype=np.float32)


def kernel(im, s, ex_s):
    nc = get_program()
    res = bass_utils.run_bass_kernel_spmd(nc, make_in_maps(im, s, ex_s),
                                          core_ids=list(range(8)))
    return finish(res.results)


if __name__ == "__main__":
    rng = np.random.default_rng(0)
    im = rng.standard_normal((N, D), dtype=np.float32)
    s = rng.standard_normal((N, D), dtype=np.float32)
    ex = rng.standard_normal((N, D), dtype=np.float32)
    print(kernel(im, s, ex))
